# revision 2
# baseline (speedup 1.0000x reference)
"""DeepSigNet Trainium2 kernel (8-core data-parallel).

Math (per batch element, matching the reference):
  path = tanh(conv1d(x[:64], w, k=3, pad=1) + b).T          # [L=512, 64]
  dx[t] = path[t+1] - path[t], t = 0..510
  S[m, j] = sum_t path[t, m] * dx[t, j]   (uncentered)
  The reference centers with prefix = path[t] - path[0]:
  S'[m, j] = S[m, j] - p0[m] * lvl1[j].
  Only the antisymmetric part of S' feeds the MLP (triu of 0.5(S'-S'^T)),
  so any symmetric difference is free: S'' = S + p0 (x) p511 == S' modulo
  a symmetric matrix.  fc1 weights are host-permuted/antisymmetrized so fc1
  consumes [S''-cols | lvl1 | static | pooled | 1] directly.

Device layout (per core, 16 batch elems):
  FT [128, 576] sbuf: 36 K-tiles of 16 columns (one col per batch elem).
    tiles 0..31: FT[p<64, 16t+b] = S''_b[t, p]; FT[p>=64] = S''_b[32+t, p-64]
    tile 32: p<64 lvl1[p]; p>=64 static chan p+1 (65..128)
    tile 33: static chan 129+p;  tile 34: static chan 257+p
    tile 35: p0 pooled; p1 const-1 (fc1 bias); p2..64 static 385..447; pad 0
  MLP [b, h] orientation with PE transposes between layers; biases for
  fc2/fc3 enter via rank-1 matmuls with a ones row.

Precision: all heavy matmul operands are bf16 (1 cyc/row on PE, half the
weight HBM traffic).  Accumulation stays fp32 in PSUM; the path and dx are
computed in fp32 and only the matmul *operands* are rounded to bf16, so
the dx cancellation keeps full precision.

I/O: everything the device reads is packed host-side into TWO flat blobs
(one bf16: conv weights, fc1/fc2/fc3 weights, conv-input slices; one f32:
constants, pre-transposed static features, pooled-max channel, biases).
Fewer PJRT operands per call = less relay dispatch overhead, and the
static features land in SBUF in their final FT layout with one block DMA
instead of five element-strided transposing DMAs.
"""

import os
import numpy as np

B, C_IN, C_OUT, L = 128, 64, 64, 512
POST, HID, OUT_DIM = 384, 1024, 128
NCORES = 8
BPC = B // NCORES   # 16
NT1 = 36            # fc1 K-tiles
D1 = NT1 * 128      # 4608 padded fc1 input dim
XW = 514            # per-elem column block in XALL (2 shifted copies + pads)
W1CHUNK = int(os.environ.get("DSN_W1CHUNK", "6"))  # fc1 K-tiles per weight DMA

# f32 blob layout (flat offsets, in elements)
CF_BLOB = 0                      # [128, 400]: cbias 256 | idn 128 | onescols 16
CF_STAT = CF_BLOB + 128 * 400    # [128, 64]: ft[:, 512:576] statics image
CF_XM = CF_STAT + 128 * 64       # [BPC, 512]: pooled-max channel
CF_B23 = CF_XM + BPC * 512       # [1, 1152]: fc2_b | fc3_b
CF_N = CF_B23 + (HID + OUT_DIM)

# bf16 blob layout
CB_WCB = 0                       # [128, 128] conv weights
CB_W1 = CB_WCB + 128 * 128       # [4608, 1024] fc1 (padded, transposed)
CB_W2 = CB_W1 + D1 * HID         # [1024, 1024] fc2 transposed
CB_W3 = CB_W2 + HID * HID        # [1024, 128] fc3 transposed
CB_XC = CB_W3 + HID * OUT_DIM    # [BPC, 64, 512] conv input slice
CB_N = CB_XC + BPC * C_IN * L

_prog_cache = {}


def _build_nc():
    key = ("nc",)
    if key in _prog_cache:
        return _prog_cache[key]

    import concourse.bass as bass
    import concourse.tile as tile
    from concourse import bacc, mybir

    f32 = mybir.dt.float32
    bf16 = mybir.dt.bfloat16
    TANH = mybir.ActivationFunctionType.Tanh

    nc = bacc.Bacc(None, target_bir_lowering=False, debug=False)

    cf_d = nc.dram_tensor("cf", [CF_N], f32, kind="ExternalInput")
    cb_d = nc.dram_tensor("cb", [CB_N], bf16, kind="ExternalInput")
    out_d = nc.dram_tensor("out", [BPC, OUT_DIM], f32, kind="ExternalOutput")

    cfa = cf_d.ap()
    cba = cb_d.ap()
    outa = out_d.ap()

    with tile.TileContext(nc) as tc:
        with (
            tc.tile_pool(name="const", bufs=1) as constp,
            tc.tile_pool(name="big", bufs=1) as bigp,
            tc.tile_pool(name="cvps", bufs=int(os.environ.get("DSN_CVBUFS", "3")), space="PSUM") as cvpsp,
            tc.tile_pool(name="smallps", bufs=int(os.environ.get("DSN_SMBUFS", "3")), space="PSUM") as smallps,
            tc.tile_pool(name="xg", bufs=int(os.environ.get("DSN_XGBUFS", "3"))) as xgp,
            tc.tile_pool(name="ptg", bufs=int(os.environ.get("DSN_GBUFS", "2"))) as ptgp,
            tc.tile_pool(name="ptshg", bufs=int(os.environ.get("DSN_GBUFS", "2"))) as ptshgp,
            tc.tile_pool(name="ptbg", bufs=int(os.environ.get("DSN_GBUFS", "2"))) as ptbgp,
            tc.tile_pool(name="ddg", bufs=int(os.environ.get("DSN_GBUFS", "2"))) as ddgp,
            tc.tile_pool(name="prow", bufs=2) as prowp,
            tc.tile_pool(name="wstream", bufs=int(os.environ.get("DSN_WBUFS", "5"))) as wsp,
            tc.tile_pool(name="wstream2", bufs=2) as wsp2,
            tc.tile_pool(name="mlpps", bufs=2, space="PSUM") as mlpps,
            tc.tile_pool(name="act", bufs=1) as actp,
        ):
            # --- constants (one blob DMA) ---
            blob = constp.tile([128, 400], f32)
            nc.scalar.dma_start(
                blob[:], cfa[CF_BLOB:CF_STAT].rearrange("(p c) -> p c", c=400))
            cbb = blob[:, 0:256]
            idn = blob[:, 256:384]
            ocst = blob[:, 384:400]
            wcb = constp.tile([128, 128], bf16)
            nc.scalar.dma_start(
                wcb[:], cba[CB_WCB:CB_W1].rearrange("(p c) -> p c", c=128))
            b23 = constp.tile([1, HID + OUT_DIM], f32)
            nc.scalar.dma_start(
                b23[:], cfa[CF_B23:CF_N].rearrange("(p c) -> p c", c=HID + OUT_DIM))
            b2s = b23[:, 0:HID]
            b3s = b23[:, HID:HID + OUT_DIM]
            ones16 = constp.tile([1, 16], f32)
            nc.gpsimd.memset(ones16[:, :], 1.0)
            ones128 = constp.tile([1, 128], f32)
            nc.gpsimd.memset(ones128[:, :], 1.0)
            idnb = constp.tile([128, 128], bf16)
            nc.vector.tensor_copy(idnb[:, :], idn)

            # --- persistent tensors ---
            ft = bigp.tile([128, NT1 * BPC], f32)          # [128, 576]
            ftr = ft[:].rearrange("p (t c) -> p t c", c=BPC)
            # statics image: covers ft[:, 512:576] (zeros where lvl1 /
            # pooled land later; ones row for the fc1 bias baked in)
            nc.scalar.dma_start(
                ft[:, 512:576],
                cfa[CF_STAT:CF_XM].rearrange("(p c) -> p c", c=64))

            # ===== front-end, pipelined in groups of GE elems =====
            GE = int(os.environ.get("DSN_GE", "8"))
            ocstr = ocst[:].rearrange("p (e b) -> p e b", b=4)
            for g in range(BPC // GE):
                e0 = GE * g
                xg = xgp.tile([128, GE * XW], bf16)
                xgr = xg[:].rearrange("p (e w) -> p e w", w=XW)
                xslice = cba[CB_XC + e0 * C_IN * L:
                             CB_XC + (e0 + GE) * C_IN * L]
                nc.sync.dma_start(
                    xgr[0:64, :, 2:514],
                    xslice.rearrange("(e c l) -> c e l", c=C_IN, l=L))
                nc.sync.dma_start(
                    xgr[64:128, :, 1:513],
                    xslice.rearrange("(e c l) -> c e l", c=C_IN, l=L))
                nc.gpsimd.memset(xgr[0:64, :, 1:2], 0.0)
                nc.gpsimd.memset(xgr[64:128, :, 513:514], 0.0)

                # per-elem 4 blocks of 65 cols: 64 path chans + a ones column
                # (telescopes sum(dx) = lvl1 inside the S matmuls; the zero at
                # (block 3, row 127) excludes the virtual dd row 511)
                ptg = ptgp.tile([128, GE * 260], f32)
                pt4 = ptg[:].rearrange("p (e b c) -> p e b c", b=4, c=65)
                for eh in range(0, GE, 4):
                    nc.vector.tensor_copy(
                        pt4[:, eh:eh + min(4, GE), :, 64],
                        ocstr[:, 0:min(4, GE), :])
                for i in range(GE):
                    xo = XW * i
                    cv = cvpsp.tile([128, 256], f32)
                    # conv bias as the rank-1 matmul opening the group; the
                    # 8 conv matmuls accumulate onto it and tanh reads PSUM
                    # directly (no DVE add on the latency chain)
                    nc.tensor.matmul(
                        cv[:, 0:256], ones128[:, :], cbb[0:1, 0:256],
                        start=True, stop=False)
                    for lt in range(4):
                        nc.tensor.matmul(
                            cv[:, 64 * lt:64 * lt + 64],
                            xg[:, xo + 128 * lt + 1:xo + 128 * lt + 129],
                            wcb[:, 0:64],
                            start=False, stop=False)
                        nc.tensor.matmul(
                            cv[:, 64 * lt:64 * lt + 64],
                            xg[64:128, xo + 128 * lt + 2:xo + 128 * lt + 130],
                            wcb[64:128, 64:128],
                            start=False, stop=(lt == 3))
                    nc.scalar.activation(
                        pt4[:, i, :, 0:64],
                        cv[:].rearrange("p (b c) -> p b c", c=64), TANH)

                # shifted path + dx (partition shifts go through DMA)
                ptshg = ptshgp.tile([128, GE * 260], f32)
                psh4 = ptshg[:].rearrange("p (e b c) -> p e b c", b=4, c=65)
                nc.sync.dma_start(ptshg[0:127, :], ptg[1:128, :])
                nc.sync.dma_start(
                    psh4[127:128, :, 0:3, :], pt4[0:1, :, 1:4, :])
                # virtual row 511: dd = 0 there (p0 (x) p511 correction enters
                # as the rank-1 matmul that OPENS each S accumulation group)
                nc.sync.dma_start(
                    psh4[127:128, :, 3, 0:65], pt4[127:128, :, 3, 0:65])
                prowg = prowp.tile([1, GE * 65], f32)
                prowr = prowg[:].rearrange("p (e c) -> p e c", c=65)
                nc.sync.dma_start(prowr[:, :, 0:64], pt4[127:128, :, 3, 0:64])
                nc.gpsimd.memset(prowr[:, :, 64:65], 0.0)
                ptb = ptbgp.tile([128, GE * 260], bf16)
                nc.vector.tensor_copy(ptb[:, :], ptg[:, :])
                ddb = ddgp.tile([128, GE * 260], bf16)
                nc.vector.tensor_sub(ddb[:, :], ptshg[:, :], ptg[:, :])

                # log-signature S'' per elem; the p0 (x) p511 rank-1 term goes
                # first so the accumulation group closes on the true last
                # matmul (no post-stop writes racing with the PSUM readers)
                for i in range(GE):
                    e = e0 + i
                    po = 260 * i
                    st = smallps.tile([128, 65], f32, tag="sm", name="st")
                    nc.tensor.matmul(
                        st[0:64, 0:65], ptg[0:1, po:po + 64],
                        prowg[0:1, 65 * i:65 * i + 65],
                        start=True, stop=False)
                    for t in range(4):
                        nc.tensor.matmul(
                            st[0:64, :],
                            ddb[:, po + 65 * t:po + 65 * t + 64],
                            ptb[:, po + 65 * t:po + 65 * t + 65],
                            start=False, stop=(t == 3))
                    nc.vector.tensor_copy(ftr[0:64, 0:32, e], st[0:64, 0:32])
                    nc.vector.tensor_copy(ftr[64:128, 0:32, e], st[0:64, 32:64])
                    nc.vector.tensor_copy(
                        ft[0:64, 512 + e:513 + e], st[0:64, 64:65])

            # ======== pooled max ========
            xm = actp.tile([BPC, 512], f32)
            nc.scalar.dma_start(
                xm[:, :], cfa[CF_XM:CF_B23].rearrange("(b l) -> b l", l=512))
            pxm = actp.tile([BPC, 1], f32)
            nc.vector.reduce_max(pxm[:, :], xm[:, :],
                                 axis=bass.mybir.AxisListType.X)
            pxt = smallps.tile([128, 65], f32, tag="sm", name="pxt")
            nc.tensor.transpose(pxt[0:1, 0:BPC], pxm[:, :], idn[0:BPC, 0:BPC])
            nc.vector.tensor_copy(ft[0:1, 560:560 + BPC], pxt[0:1, 0:BPC])

            # ======================= MLP =======================
            ftc = actp.tile([128, NT1 * BPC], bf16)
            nc.vector.tensor_copy(ftc[:, :], ft[:, :])
            ftmm = ftc[:].rearrange("p (t c) -> p t c", c=BPC)

            # fc1: H1[b, h] = FT.T @ W1T, weights streamed in W1CHUNK K-tiles
            h1ps = [mlpps.tile([BPC, 512], f32, tag="hps", name=f"h1ps{i}")
                    for i in range(2)]
            nchunk = NT1 // W1CHUNK
            w1tiles = []
            for ck in range(nchunk):
                w1s = wsp.tile([128, W1CHUNK * HID], bf16, tag="ws")
                nc.sync.dma_start(
                    w1s[:].rearrange("p (t h) -> p t h", h=HID),
                    cba[CB_W1 + 128 * W1CHUNK * ck * HID:
                        CB_W1 + 128 * W1CHUNK * (ck + 1) * HID]
                    .rearrange("(t p h) -> p t h", p=128, h=HID))
                w1tiles.append(w1s)
            h1 = actp.tile([BPC, HID], bf16)
            h1t = actp.tile([128, 128], bf16)
            for nt in range(2):
                for ck in range(nchunk):
                    for t in range(W1CHUNK):
                        kt = W1CHUNK * ck + t
                        nc.tensor.matmul(
                            h1ps[nt][:, :],
                            ftmm[:, kt, :],
                            w1tiles[ck][:, HID * t + 512 * nt:
                                        HID * t + 512 * nt + 512],
                            start=(kt == 0), stop=(kt == NT1 - 1))
                nc.vector.tensor_relu(h1[:, 512 * nt:512 * nt + 512],
                                      h1ps[nt][:, :])
                for i in range(4 * nt, 4 * nt + 4):
                    tp = smallps.tile([128, 65], bf16, tag="sm", name="tp")
                    nc.tensor.transpose(
                        tp[:, 0:BPC], h1[:, 128 * i:128 * i + 128],
                        idnb[0:BPC, 0:BPC])
                    nc.vector.tensor_copy(
                        h1t[:, 16 * i:16 * i + 16], tp[:, 0:BPC])

            # fc2
            h2ps = [mlpps.tile([BPC, 512], f32, tag="hps", name=f"h2ps{i}")
                    for i in range(2)]
            for ck in range(2):
                w2s = wsp2.tile([128, 4 * HID], bf16, name="w2s")
                nc.sync.dma_start(
                    w2s[:].rearrange("p (t h) -> p t h", h=HID),
                    cba[CB_W2 + 512 * ck * HID:CB_W2 + 512 * (ck + 1) * HID]
                    .rearrange("(t p h) -> p t h", p=128, h=HID))
                for t in range(4):
                    kt = 4 * ck + t
                    for nt in range(2):
                        nc.tensor.matmul(
                            h2ps[nt][:, :],
                            h1t[:, 16 * kt:16 * kt + 16],
                            w2s[:, HID * t + 512 * nt:
                                512 * nt + HID * t + 512],
                            start=(kt == 0), stop=False)
            h2 = actp.tile([BPC, HID], bf16)
            h2t = actp.tile([128, 128], bf16)
            for nt in range(2):
                nc.tensor.matmul(
                    h2ps[nt][:, :], ones16[:, :],
                    b2s[0:1, 512 * nt:512 * nt + 512],
                    start=False, stop=True)
                nc.vector.tensor_relu(h2[:, 512 * nt:512 * nt + 512],
                                      h2ps[nt][:, :])
                for i in range(4 * nt, 4 * nt + 4):
                    tp = smallps.tile([128, 65], bf16, tag="sm", name="tp")
                    nc.tensor.transpose(
                        tp[:, 0:BPC], h2[:, 128 * i:128 * i + 128],
                        idnb[0:BPC, 0:BPC])
                    nc.vector.tensor_copy(
                        h2t[:, 16 * i:16 * i + 16], tp[:, 0:BPC])

            # fc3
            w3s = actp.tile([128, HID], bf16)
            nc.scalar.dma_start(
                w3s[:].rearrange("p (t o) -> p t o", o=OUT_DIM),
                cba[CB_W3:CB_XC].rearrange("(t p o) -> p t o",
                                           p=128, o=OUT_DIM))
            ops = mlpps.tile([BPC, 512], f32, tag="hps")
            for kt in range(8):
                nc.tensor.matmul(
                    ops[:, 0:OUT_DIM],
                    h2t[:, 16 * kt:16 * kt + 16],
                    w3s[:, OUT_DIM * kt:OUT_DIM * kt + OUT_DIM],
                    start=(kt == 0), stop=False)
            nc.tensor.matmul(
                ops[:, 0:OUT_DIM], ones16[:, :], b3s, start=False, stop=True)
            outsb = actp.tile([BPC, OUT_DIM], f32)
            nc.vector.tensor_copy(outsb[:, :], ops[:, 0:OUT_DIM])
            nc.scalar.dma_start(outa[:, :], outsb[:, :])

    nc.compile()
    _prog_cache[key] = nc
    return nc


def _host_weights(conv_w, conv_b, fc1_w, fc1_b, fc2_w, fc2_b, fc3_w, fc3_b):
    """Shared (replicated) halves of the two blobs."""
    import ml_dtypes
    f = np.float32
    bf = ml_dtypes.bfloat16
    conv_w = np.asarray(conv_w, f)
    fc1_w = np.asarray(fc1_w, f)

    blob = np.zeros((128, 400), f)
    blob[:, 0:256] = np.tile(np.asarray(conv_b, f)[None, :], (128, 4))
    blob[:, 256:384] = np.eye(128, dtype=f)
    blob[:, 384:400] = 1.0
    blob[127, 384 + 3::4] = 0.0

    # conv weights [i, tap*64+o]; taps (0,1) stacked for the K=128 matmul at
    # cols 0:64 of partitions (0:64, 64:128); tap 2 at rows 64:128 cols 64:128
    wtap = conv_w.transpose(1, 2, 0)  # [i, k, o]
    wcb = np.zeros((128, 128), f)
    wcb[0:64, 0:64] = wtap[:, 0, :]
    wcb[64:128, 0:64] = wtap[:, 1, :]
    wcb[64:128, 64:128] = wtap[:, 2, :]

    wfull = np.zeros((HID, 64, 64), f)
    iu, ju = np.triu_indices(64, 1)
    wtri = fc1_w[:, 64:2080]
    wfull[:, iu, ju] = 0.5 * wtri
    wfull[:, ju, iu] = -0.5 * wtri

    w1t = np.zeros((D1, HID), f)
    for t in range(32):
        w1t[128 * t:128 * t + 64, :] = wfull[:, t, :].T
        w1t[128 * t + 64:128 * t + 128, :] = wfull[:, 32 + t, :].T
    # tile 32: lvl1 (p<64), static chans 65..128 (p>=64)
    w1t[4096:4160, :] = fc1_w[:, 0:64].T
    w1t[4160:4224, :] = fc1_w[:, 2081:2145].T
    # tiles 33, 34: static chans 129..384
    w1t[4224:4352, :] = fc1_w[:, 2145:2273].T
    w1t[4352:4480, :] = fc1_w[:, 2273:2401].T
    # tile 35: p0 pooled, p1 const-1 -> fc1 bias, p2..64 static 385..447
    w1t[4480, :] = fc1_w[:, 2080]
    w1t[4481, :] = np.asarray(fc1_b, f)
    w1t[4482:4545, :] = fc1_w[:, 2401:2464].T

    w2t = np.ascontiguousarray(np.asarray(fc2_w, f).T)
    w3t = np.ascontiguousarray(np.asarray(fc3_w, f).T)
    b23 = np.concatenate(
        [np.asarray(fc2_b, f), np.asarray(fc3_b, f)])[None, :]

    cb_shared = np.empty(CB_XC, bf)
    cb_shared[CB_WCB:CB_W1] = wcb.reshape(-1).astype(bf)
    cb_shared[CB_W1:CB_W2] = w1t.reshape(-1).astype(bf)
    cb_shared[CB_W2:CB_W3] = w2t.reshape(-1).astype(bf)
    cb_shared[CB_W3:CB_XC] = w3t.reshape(-1).astype(bf)
    return blob, b23, cb_shared


def make_in_maps(x, conv_w, conv_b, fc1_w, fc1_b, fc2_w, fc2_b, fc3_w, fc3_b):
    import ml_dtypes
    f = np.float32
    bf = ml_dtypes.bfloat16
    blob, b23, cb_shared = _host_weights(
        conv_w, conv_b, fc1_w, fc1_b, fc2_w, fc2_b, fc3_w, fc3_b)
    x = np.asarray(x, f)
    xc = x[:, 0:C_IN, :].astype(bf)
    xm = np.ascontiguousarray(x[:, C_IN, :])
    xs = np.ascontiguousarray(x[:, C_IN + 1:, 0])   # [B, POST-1]
    in_maps = []
    for c in range(NCORES):
        sl = slice(BPC * c, BPC * (c + 1))
        xsl = xs[sl]                                 # [BPC, 383]
        # statics image of ft[:, 512:576]: 4 column-blocks of 16 (one col
        # per elem); zeros where lvl1 / pooled are device-computed
        stat = np.zeros((128, 64), f)
        stat[64:128, 0:16] = xsl[:, 0:64].T
        stat[:, 16:32] = xsl[:, 64:192].T
        stat[:, 32:48] = xsl[:, 192:320].T
        stat[1, 48:64] = 1.0
        stat[2:65, 48:64] = xsl[:, 320:383].T
        cf = np.empty(CF_N, f)
        cf[CF_BLOB:CF_STAT] = blob.reshape(-1)
        cf[CF_STAT:CF_XM] = stat.reshape(-1)
        cf[CF_XM:CF_B23] = xm[sl].reshape(-1)
        cf[CF_B23:CF_N] = b23.reshape(-1)
        cb = np.empty(CB_N, bf)
        cb[0:CB_XC] = cb_shared
        cb[CB_XC:CB_N] = np.ascontiguousarray(xc[sl]).reshape(-1)
        in_maps.append(dict(cf=cf, cb=cb))
    return in_maps


def kernel(x, conv_w, conv_b, fc1_w, fc1_b, fc2_w, fc2_b, fc3_w, fc3_b):
    from concourse.bass_utils import run_bass_kernel_spmd

    nc = _build_nc()
    in_maps = make_in_maps(x, conv_w, conv_b, fc1_w, fc1_b, fc2_w, fc2_b,
                           fc3_w, fc3_b)
    res = run_bass_kernel_spmd(nc, in_maps, list(range(NCORES)))
    out = np.concatenate([res.results[c]["out"] for c in range(NCORES)], axis=0)
    return out.astype(np.float32)
